# revision 1
# baseline (speedup 1.0000x reference)
"""MoE LoRA adapter layer (top-2 routed, E=8 experts, R=16) on 8 TRN2 NeuronCores.

Strategy: data-parallel over batch B=32 -> 4 batches/core; router + LoRA
weights replicated (tiny). Key observation: E*R = 128 = partition width, so
the per-expert LoRA down/up projections stack into two dense matmuls:
    P1T[er, t] = D_all[er, :] @ x[t, :]^T          (contract H=1024)
    w[t, h]    = sum_er gate[b(t), e(er)] * P1T[er, t] * U_all[er, h]
The expert sum IS the matmul contraction; gates (exactly 0 off the top-2)
are folded in by scaling P1T columns. out = x + w.

Layout: x is shipped bf16 (matmuls are bf16 anyway); two consecutive tokens
share an SBUF partition so every DMA descriptor is a 4 KiB contiguous run.
Loads ride SWDGE (gpsimd), stores ride HWDGE (sync) - independent queues.
Gates are computed on-device in fp32 (exact top-2) from a tiny f32 cls copy.
"""

import sys

if "/opt/trn_rl_repo" not in sys.path:
    sys.path.insert(0, "/opt/trn_rl_repo")

import numpy as np
import ml_dtypes

import concourse.bass as bass
import concourse.tile as tile
from concourse import bacc, mybir
from concourse.bass_utils import run_bass_kernel_spmd

B, L, H = 32, 512, 1024
E, R, TOP_K = 8, 16, 2
N_CORES = 8
NB = B // N_CORES          # batches per core = 4
T = NB * L                 # tokens per core = 2048
P = 128                    # partitions
NK = H // P                # H k-tiles = 8
NS = 2                     # half-chunks per batch (256 tokens each)

F32 = mybir.dt.float32
BF16 = mybir.dt.bfloat16
BF16_NP = ml_dtypes.bfloat16

_COMPILED = None


def _build():
    """Build + compile the single-core program (same on all 8 cores)."""
    nc = bacc.Bacc("TRN2", target_bir_lowering=False, debug=False)

    x_in = nc.dram_tensor("x_in", [T, H], BF16, kind="ExternalInput")
    cls_in = nc.dram_tensor("cls_in", [NB, H], F32, kind="ExternalInput")
    d_t = nc.dram_tensor("d_t", [P, NK * P], BF16, kind="ExternalInput")
    u_in = nc.dram_tensor("u_in", [P, H], BF16, kind="ExternalInput")
    rwt = nc.dram_tensor("rwt", [P, NK * E], F32, kind="ExternalInput")
    rep = nc.dram_tensor("rep", [E, P], F32, kind="ExternalInput")
    idn = nc.dram_tensor("idn", [P, P], F32, kind="ExternalInput")
    idnb = nc.dram_tensor("idnb", [P, P], BF16, kind="ExternalInput")
    y_out = nc.dram_tensor("y_out", [T, H], BF16, kind="ExternalOutput")

    # token t = c*512 + s*256 + 2p + j  <->  partition p, pair j, half s
    x_pair = x_in.ap().rearrange("(c s p j) h -> c s p j h", s=NS, p=P, j=2)
    y_pair = y_out.ap().rearrange("(c s p j) h -> c s p j h", s=NS, p=P, j=2)

    with tile.TileContext(nc) as tc:
        with (
            tc.tile_pool(name="wpool", bufs=1) as wpool,
            tc.tile_pool(name="xbpool", bufs=8) as xbpool,
            tc.tile_pool(name="xtpool", bufs=3) as xtpool,
            tc.tile_pool(name="p2pool", bufs=3) as p2pool,
            tc.tile_pool(name="opool", bufs=8) as opool,
            tc.tile_pool(name="gpool", bufs=1) as gpool,
            tc.tile_pool(name="tp_ps", bufs=2, space="PSUM") as tp_ps,
            tc.tile_pool(name="p1_ps", bufs=2, space="PSUM") as p1_ps,
            tc.tile_pool(name="w_ps", bufs=2, space="PSUM") as w_ps,
        ):
            # ---- small constants first (gates prologue deps) ----
            cls_nat = gpool.tile([NB, H], F32, tag="cls")
            nc.gpsimd.dma_start(cls_nat[:], cls_in.ap())
            id_sb = wpool.tile([P, P], F32, tag="idn")
            nc.sync.dma_start(id_sb[:], idn.ap())
            idb_sb = wpool.tile([P, P], BF16, tag="idnb")
            nc.sync.dma_start(idb_sb[:], idnb.ap())
            rwt_sb = wpool.tile([P, NK * E], F32, tag="rwt")
            nc.sync.dma_start(rwt_sb[:], rwt.ap())
            rep_sb = wpool.tile([E, P], F32, tag="rep")
            nc.sync.dma_start(rep_sb[:], rep.ap())
            d_sb = wpool.tile([P, NK * P], BF16, tag="d")
            nc.sync.dma_start(d_sb[:], d_t.ap())
            u_sb = wpool.tile([P, H], BF16, tag="u")
            nc.sync.dma_start(u_sb[:], u_in.ap())

            holders = {}

            def stage_prologue():
                clsT = gpool.tile([P, NK * NB], F32, tag="clsT")
                cps = w_ps.tile([P, 512], F32, tag="w")
                for k in range(NK):
                    nc.tensor.transpose(
                        cps[:, k * NB : (k + 1) * NB],
                        cls_nat[:, k * P : (k + 1) * P],
                        id_sb[0:NB, 0:NB],
                    )
                nc.vector.tensor_copy(clsT[:, 0 : NK * NB], cps[:, 0 : NK * NB])

                lg_ps = w_ps.tile([P, 512], F32, tag="w")
                for k in range(NK):
                    nc.tensor.matmul(
                        lg_ps[0:NB, 0:E],
                        clsT[:, k * NB : (k + 1) * NB],
                        rwt_sb[:, k * E : (k + 1) * E],
                        start=(k == 0),
                        stop=(k == NK - 1),
                    )
                lg = gpool.tile([NB, E], F32, tag="lg")
                nc.vector.tensor_copy(lg[:], lg_ps[0:NB, 0:E])

                # top-2 softmax per row (E=8 along free dim)
                m1 = gpool.tile([NB, 1], F32, tag="m1")
                nc.vector.reduce_max(m1[:], lg[:], axis=mybir.AxisListType.X)
                t_sb = gpool.tile([NB, E], F32, tag="t")
                nc.vector.tensor_scalar(
                    t_sb[:], lg[:], m1[:], None, op0=mybir.AluOpType.subtract
                )
                # pen = (t >= 0) * 1e30  (knocks out the argmax)
                pen = gpool.tile([NB, E], F32, tag="pen")
                nc.vector.tensor_scalar(
                    pen[:], t_sb[:], 0.0, 1e30,
                    op0=mybir.AluOpType.is_ge, op1=mybir.AluOpType.mult,
                )
                t2 = gpool.tile([NB, E], F32, tag="t2")
                nc.vector.tensor_sub(t2[:], t_sb[:], pen[:])
                m2 = gpool.tile([NB, 1], F32, tag="m2")
                nc.vector.reduce_max(m2[:], t2[:], axis=mybir.AxisListType.X)
                keep = gpool.tile([NB, E], F32, tag="keep")
                nc.vector.tensor_scalar(
                    keep[:], t_sb[:], m2[:], None, op0=mybir.AluOpType.is_ge
                )
                ex = gpool.tile([NB, E], F32, tag="ex")
                nc.scalar.activation(ex[:], t_sb[:], mybir.ActivationFunctionType.Exp)
                eg = gpool.tile([NB, E], F32, tag="eg")
                nc.vector.tensor_mul(eg[:], ex[:], keep[:])
                s_sb = gpool.tile([NB, 1], F32, tag="s")
                nc.vector.reduce_sum(s_sb[:], eg[:], axis=mybir.AxisListType.X)
                rs = gpool.tile([NB, 1], F32, tag="rs")
                nc.vector.reciprocal(rs[:], s_sb[:])
                gts = gpool.tile([NB, E], F32, tag="gts")
                nc.vector.tensor_scalar(
                    gts[:], eg[:], rs[:], None, op0=mybir.AluOpType.mult
                )

                # gatesT then replicate x16 along partitions -> gvec [128, NB]
                gt_ps = w_ps.tile([P, 512], F32, tag="w")
                nc.tensor.transpose(gt_ps[0:E, 0:NB], gts[:], id_sb[0:NB, 0:NB])
                gtT = gpool.tile([E, NB], F32, tag="gtT")
                nc.vector.tensor_copy(gtT[:], gt_ps[0:E, 0:NB])
                gv_ps = w_ps.tile([P, 512], F32, tag="w")
                nc.tensor.matmul(gv_ps[:, 0:NB], rep_sb[:], gtT[:])
                gvec = gpool.tile([P, NB], F32, tag="gvec")
                nc.vector.tensor_copy(gvec[:], gv_ps[:, 0:NB])
                holders["gvec"] = gvec

            # ---- main loop: one chunk = one batch (512 tokens) ----
            # Software-pipelined emission: transposes of chunk c, then MM1(c),
            # then MM2(c-1), so PE never stalls on a just-produced input.
            xb_tiles = {}
            xt_tiles = {}
            p2_tiles = {}

            def stage_transpose(c):
                # SWDGE loads (separate queue from HWDGE stores); 4 KiB runs
                xbs = []
                for s in range(NS):
                    xb = xbpool.tile([P, 2, H], BF16, tag="xb")
                    nc.gpsimd.dma_start(xb[:], x_pair[c, s])
                    xbs.append(xb)
                xb_tiles[c] = xbs
                xt = xtpool.tile([P, NK, 512], BF16, tag="xt")
                xt_tiles[c] = xt
                for kg in range(2):
                    for s in range(NS):
                        for j in range(2):
                            tp = tp_ps.tile([P, 4, P], BF16, tag="tpb")
                            for q in range(4):
                                k = kg * 4 + q
                                nc.tensor.transpose(
                                    tp[:, q, :],
                                    xbs[s][:, j, k * P : (k + 1) * P],
                                    idb_sb[:],
                                )
                            nc.scalar.activation(
                                xt[:, kg * 4 : (kg + 1) * 4,
                                   s * 256 + j * P : s * 256 + (j + 1) * P],
                                tp[:],
                                mybir.ActivationFunctionType.Copy,
                            )

            def stage_mm1(c):
                xt = xt_tiles[c]
                p1 = p1_ps.tile([P, 512], F32, tag="p1")
                for k in range(NK):
                    nc.tensor.matmul(
                        p1[:],
                        d_sb[:, k * P : (k + 1) * P],
                        xt[:, k, :],
                        start=(k == 0),
                        stop=(k == NK - 1),
                    )
                p2t = p2pool.tile([P, 512], BF16, tag="p2t")
                for s in range(NS):
                    nc.vector.tensor_scalar(
                        p2t[:, s * 256 : (s + 1) * 256],
                        p1[:, s * 256 : (s + 1) * 256],
                        holders["gvec"][:, c : c + 1], None,
                        op0=mybir.AluOpType.mult,
                    )
                p2_tiles[c] = p2t

            def stage_mm2(c):
                p2t = p2_tiles[c]
                xbs = xb_tiles[c]
                for s in range(NS):
                    o_sb = opool.tile([P, 2, H], BF16, tag="o")
                    for j in range(2):
                        wp = w_ps.tile([P, H], F32, tag="w")
                        for h2 in range(2):
                            nc.tensor.matmul(
                                wp[:, h2 * 512 : (h2 + 1) * 512],
                                p2t[:, s * 256 + j * P : s * 256 + (j + 1) * P],
                                u_sb[:, h2 * 512 : (h2 + 1) * 512],
                            )
                        nc.vector.tensor_add(
                            o_sb[:, j, :],
                            wp[:],
                            xbs[s][:, j, :],
                        )
                    nc.sync.dma_start(y_pair[c, s], o_sb[:])

            stage_prologue()
            for c in range(NB):
                stage_transpose(c)
                if c >= 1:
                    stage_mm1(c - 1)
                if c >= 2:
                    stage_mm2(c - 2)
            stage_mm1(NB - 1)
            stage_mm2(NB - 2)
            stage_mm2(NB - 1)

    nc.compile()
    return nc


def _weights_maps(router_w, lora_down, lora_up):
    # D_all[(e,r), h] stacked; lhsT tiles need [p, k, m] = D_all[m, k*128+p]
    d_all = lora_down.reshape(E * R, H)                       # [128, 1024]
    d_t = np.ascontiguousarray(
        d_all.T.reshape(NK, P, E * R).transpose(1, 0, 2).reshape(P, NK * P)
    ).astype(BF16_NP)
    # U_all[(e,r), h] = lora_up[e, h, r]
    u_np = np.ascontiguousarray(
        lora_up.transpose(0, 2, 1).reshape(E * R, H)
    ).astype(BF16_NP)
    # router_wT tiles [p, k, e] = router_w[e, k*128+p]
    rwt_np = np.ascontiguousarray(
        router_w.T.reshape(NK, P, E).transpose(1, 0, 2).reshape(P, NK * E)
    ).astype(np.float32)
    rep_np = np.zeros((E, P), np.float32)
    for e in range(E):
        rep_np[e, e * R : (e + 1) * R] = 1.0
    idn_np = np.eye(P, dtype=np.float32)
    return {
        "d_t": d_t, "u_in": u_np, "rwt": rwt_np, "rep": rep_np,
        "idn": idn_np, "idnb": idn_np.astype(BF16_NP),
    }


def get_compiled():
    global _COMPILED
    if _COMPILED is None:
        _COMPILED = _build()
    return _COMPILED


def make_in_maps(x, router_w, lora_down, lora_up):
    x = np.asarray(x, np.float32)
    w_maps = _weights_maps(
        np.asarray(router_w, np.float32),
        np.asarray(lora_down, np.float32),
        np.asarray(lora_up, np.float32),
    )
    in_maps = []
    for i in range(N_CORES):
        shard = np.ascontiguousarray(
            x[i * NB : (i + 1) * NB].reshape(T, H)
        ).astype(BF16_NP)
        cls_shard = np.ascontiguousarray(x[i * NB : (i + 1) * NB, 0, :])
        in_maps.append({"x_in": shard, "cls_in": cls_shard, **w_maps})
    return in_maps


def kernel(x, router_w, lora_down, lora_up):
    nc = get_compiled()
    in_maps = make_in_maps(x, router_w, lora_down, lora_up)
    res = run_bass_kernel_spmd(nc, in_maps, core_ids=list(range(N_CORES)))
    out = np.empty((B, L, H), np.float32)
    for i in range(N_CORES):
        out[i * NB : (i + 1) * NB] = np.asarray(
            res.results[i]["y_out"], np.float32
        ).reshape(NB, L, H)
    return out



# revision 2
# speedup vs baseline: 1.1720x; 1.1720x over previous
"""MoE LoRA adapter layer (top-2 routed, E=8 experts, R=16) on 8 TRN2 NeuronCores.

Strategy: data-parallel over batch B=32 -> 4 batches/core; router + LoRA
weights replicated (tiny). E*R = 128 = partition width, so the per-expert
LoRA down/up projections stack into two dense matmuls:
    P1[er, t] = D_all[er, :] @ x[t, :]^T          (contract H=1024)
    wT[h, t]  = sum_er U_all[er, h] * (gate[b(t), e(er)] * P1[er, t])
The expert sum IS the matmul contraction; gates (exactly 0 off the top-2)
are folded in by scaling P1 columns. out = x + w.

Layout: x is shipped ALREADY TRANSPOSED (h-major) from the host, so the
kernel needs zero PE transposes: MM1 consumes xT directly and MM2 produces
outT in the same h-major layout the store expects. Per chunk (= one batch,
512 tokens) the input slice is one 8 KiB contiguous run per partition.
PSUM->SBUF eviction of the result (the residual add) is split between the
vector and scalar engines so neither becomes the bottleneck. Gates are
computed on-device in fp32 (exact top-2) from a tiny pre-transposed cls.
"""

import sys

if "/opt/trn_rl_repo" not in sys.path:
    sys.path.insert(0, "/opt/trn_rl_repo")

import numpy as np
import ml_dtypes

import concourse.bass as bass
import concourse.tile as tile
from concourse import bacc, mybir
from concourse.bass_utils import run_bass_kernel_spmd

B, L, H = 32, 512, 1024
E, R, TOP_K = 8, 16, 2
N_CORES = 8
NB = B // N_CORES          # batches per core = 4
T = NB * L                 # tokens per core = 2048
P = 128                    # partitions
NK = H // P                # H k-tiles = 8
C = NB                     # chunks per core (one batch = 512 tokens each)
CT = L                     # tokens per chunk

# eviction k-slices that go PSUM->vector directly; the rest take the
# scalar-copy + bf16-add path so scalar shares the eviction load
VEC_DIRECT = (0, 3, 6)

F32 = mybir.dt.float32
BF16 = mybir.dt.bfloat16
BF16_NP = ml_dtypes.bfloat16

_COMPILED = None


def _build():
    """Build + compile the single-core program (same on all 8 cores)."""
    nc = bacc.Bacc("TRN2", target_bir_lowering=False, debug=False)

    x_in = nc.dram_tensor("x_in", [P, C * NK * CT], BF16, kind="ExternalInput")
    cls_t = nc.dram_tensor("cls_t", [P, NK * NB], F32, kind="ExternalInput")
    d_t = nc.dram_tensor("d_t", [P, NK * P], BF16, kind="ExternalInput")
    u_in = nc.dram_tensor("u_in", [P, H], BF16, kind="ExternalInput")
    rwt = nc.dram_tensor("rwt", [P, NK * E], F32, kind="ExternalInput")
    rep = nc.dram_tensor("rep", [E, P], F32, kind="ExternalInput")
    id8 = nc.dram_tensor("id8", [8, 8], F32, kind="ExternalInput")
    y_out = nc.dram_tensor("y_out", [P, C * NK * CT], BF16, kind="ExternalOutput")

    # (p, c, k, t): chunk c, h-tile k, token t -> xT[k*128+p, c*512+t]
    x_ap = x_in.ap().rearrange("p (c f) -> c p f", c=C)
    # stores go out per (chunk, k-pair): 2 KiB contiguous per partition
    y_ap = y_out.ap().rearrange("p (c g f) -> c g p f", c=C, g=NK // 2)

    with tile.TileContext(nc) as tc:
        with (
            tc.tile_pool(name="wpool", bufs=1) as wpool,
            tc.tile_pool(name="gpool", bufs=1) as gpool,
            tc.tile_pool(name="xpool", bufs=C) as xpool,
            tc.tile_pool(name="opool", bufs=3) as opool,
            tc.tile_pool(name="p2pool", bufs=2) as p2pool,
            tc.tile_pool(name="wbpool", bufs=4) as wbpool,
            tc.tile_pool(name="p1_ps", bufs=2, space="PSUM") as p1_ps,
            tc.tile_pool(name="w_ps", bufs=4, space="PSUM") as w_ps,
            tc.tile_pool(name="g_ps", bufs=1, space="PSUM") as g_ps,
        ):
            # ---- loads: tiny gate inputs first, then streaming x chunks ----
            cls_sb = gpool.tile([P, NK * NB], F32, tag="cls")
            nc.gpsimd.dma_start(cls_sb[:], cls_t.ap())
            rwt_sb = wpool.tile([P, NK * E], F32, tag="rwt")
            nc.sync.dma_start(rwt_sb[:], rwt.ap())
            rep_sb = wpool.tile([E, P], F32, tag="rep")
            nc.sync.dma_start(rep_sb[:], rep.ap())
            id_sb = wpool.tile([8, 8], F32, tag="id8")
            nc.sync.dma_start(id_sb[:], id8.ap())
            d_sb = wpool.tile([P, NK * P], BF16, tag="d")
            nc.sync.dma_start(d_sb[:], d_t.ap())
            u_sb = wpool.tile([P, H], BF16, tag="u")
            nc.sync.dma_start(u_sb[:], u_in.ap())

            x_tiles = []
            for c in range(C):
                xb = xpool.tile([P, NK * CT], BF16, tag="xb")
                nc.gpsimd.dma_start(xb[:], x_ap[c])
                x_tiles.append(xb)

            holders = {}

            def stage_prologue():
                # logits [NB, E] = cls @ router_w^T, contracted over H
                lg_ps = g_ps.tile([P, 512], F32, tag="g")
                for k in range(NK):
                    nc.tensor.matmul(
                        lg_ps[0:NB, 0:E],
                        cls_sb[:, k * NB : (k + 1) * NB],
                        rwt_sb[:, k * E : (k + 1) * E],
                        start=(k == 0),
                        stop=(k == NK - 1),
                    )
                lg = gpool.tile([NB, E], F32, tag="lg")
                nc.vector.tensor_copy(lg[:], lg_ps[0:NB, 0:E])

                # top-2 softmax per row (E=8 along free dim)
                m1 = gpool.tile([NB, 1], F32, tag="m1")
                nc.vector.reduce_max(m1[:], lg[:], axis=mybir.AxisListType.X)
                t_sb = gpool.tile([NB, E], F32, tag="t")
                nc.vector.tensor_scalar(
                    t_sb[:], lg[:], m1[:], None, op0=mybir.AluOpType.subtract
                )
                # pen = (t >= 0) * 1e30  (knocks out the argmax)
                pen = gpool.tile([NB, E], F32, tag="pen")
                nc.vector.tensor_scalar(
                    pen[:], t_sb[:], 0.0, 1e30,
                    op0=mybir.AluOpType.is_ge, op1=mybir.AluOpType.mult,
                )
                t2 = gpool.tile([NB, E], F32, tag="t2")
                nc.vector.tensor_sub(t2[:], t_sb[:], pen[:])
                m2 = gpool.tile([NB, 1], F32, tag="m2")
                nc.vector.reduce_max(m2[:], t2[:], axis=mybir.AxisListType.X)
                keep = gpool.tile([NB, E], F32, tag="keep")
                nc.vector.tensor_scalar(
                    keep[:], t_sb[:], m2[:], None, op0=mybir.AluOpType.is_ge
                )
                ex = gpool.tile([NB, E], F32, tag="ex")
                nc.scalar.activation(ex[:], t_sb[:], mybir.ActivationFunctionType.Exp)
                eg = gpool.tile([NB, E], F32, tag="eg")
                nc.vector.tensor_mul(eg[:], ex[:], keep[:])
                s_sb = gpool.tile([NB, 1], F32, tag="s")
                nc.vector.reduce_sum(s_sb[:], eg[:], axis=mybir.AxisListType.X)
                rs = gpool.tile([NB, 1], F32, tag="rs")
                nc.vector.reciprocal(rs[:], s_sb[:])
                gts = gpool.tile([NB, E], F32, tag="gts")
                nc.vector.tensor_scalar(
                    gts[:], eg[:], rs[:], None, op0=mybir.AluOpType.mult
                )

                # gatesT then replicate x16 along partitions -> gvec [128, NB]
                gt_ps = g_ps.tile([P, 512], F32, tag="g")
                nc.tensor.transpose(gt_ps[0:E, 0:NB], gts[:], id_sb[0:NB, 0:NB])
                gtT = gpool.tile([E, NB], F32, tag="gtT")
                nc.vector.tensor_copy(gtT[:], gt_ps[0:E, 0:NB])
                gv_ps = g_ps.tile([P, 512], F32, tag="g")
                nc.tensor.matmul(gv_ps[:, 0:NB], rep_sb[:], gtT[:])
                gvec = gpool.tile([P, NB], F32, tag="gvec")
                nc.vector.tensor_copy(gvec[:], gv_ps[:, 0:NB])
                holders["gvec"] = gvec

            p2_tiles = {}

            def stage_mm1(c):
                p1 = p1_ps.tile([P, CT], F32, tag="p1")
                for k in range(NK):
                    nc.tensor.matmul(
                        p1[:],
                        d_sb[:, k * P : (k + 1) * P],
                        x_tiles[c][:, k * CT : (k + 1) * CT],
                        start=(k == 0),
                        stop=(k == NK - 1),
                    )
                p2 = p2pool.tile([P, CT], BF16, tag="p2")
                nc.vector.tensor_scalar(
                    p2[:], p1[:], holders["gvec"][:, c : c + 1], None,
                    op0=mybir.AluOpType.mult,
                )
                p2_tiles[c] = p2

            def stage_mm2(c):
                o_sb = opool.tile([P, NK * CT], BF16, tag="o")
                for k in range(NK):
                    wps = w_ps.tile([P, CT], F32, tag="w")
                    nc.tensor.matmul(
                        wps[:], u_sb[:, k * P : (k + 1) * P], p2_tiles[c][:]
                    )
                    x_k = x_tiles[c][:, k * CT : (k + 1) * CT]
                    o_k = o_sb[:, k * CT : (k + 1) * CT]
                    if k in VEC_DIRECT:
                        nc.vector.tensor_add(o_k, wps[:], x_k)
                    else:
                        wb = wbpool.tile([P, CT], BF16, tag="wb")
                        nc.scalar.activation(
                            wb[:], wps[:], mybir.ActivationFunctionType.Copy
                        )
                        nc.vector.tensor_add(o_k, wb[:], x_k)
                    if k % 2 == 1:
                        nc.sync.dma_start(
                            y_ap[c, k // 2], o_sb[:, (k - 1) * CT : (k + 1) * CT]
                        )

            stage_prologue()
            stage_mm1(0)
            for c in range(1, C):
                stage_mm1(c)
                stage_mm2(c - 1)
            stage_mm2(C - 1)

    nc.compile()
    return nc


def _weights_maps(router_w, lora_down, lora_up):
    # D_all[(e,r), h] stacked; lhsT tiles need [p, k, m] = D_all[m, k*128+p]
    d_all = lora_down.reshape(E * R, H)                       # [128, 1024]
    d_t = np.ascontiguousarray(
        d_all.T.reshape(NK, P, E * R).transpose(1, 0, 2).reshape(P, NK * P)
    ).astype(BF16_NP)
    # U_all[(e,r), h] = lora_up[e, h, r]
    u_np = np.ascontiguousarray(
        lora_up.transpose(0, 2, 1).reshape(E * R, H)
    ).astype(BF16_NP)
    # router_wT tiles [p, k, e] = router_w[e, k*128+p]
    rwt_np = np.ascontiguousarray(
        router_w.T.reshape(NK, P, E).transpose(1, 0, 2).reshape(P, NK * E)
    ).astype(np.float32)
    rep_np = np.zeros((E, P), np.float32)
    for e in range(E):
        rep_np[e, e * R : (e + 1) * R] = 1.0
    return {
        "d_t": d_t, "u_in": u_np, "rwt": rwt_np, "rep": rep_np,
        "id8": np.eye(8, dtype=np.float32),
    }


def get_compiled():
    global _COMPILED
    if _COMPILED is None:
        _COMPILED = _build()
    return _COMPILED


def make_in_maps(x, router_w, lora_down, lora_up):
    x = np.asarray(x, np.float32)
    w_maps = _weights_maps(
        np.asarray(router_w, np.float32),
        np.asarray(lora_down, np.float32),
        np.asarray(lora_up, np.float32),
    )
    in_maps = []
    for i in range(N_CORES):
        xs = x[i * NB : (i + 1) * NB]                         # [C, CT, H]
        # (p, c, k, t) <- xs[c, t, k*128+p]
        xtd = np.ascontiguousarray(
            xs.reshape(C, CT, NK, P).transpose(3, 0, 2, 1).reshape(P, C * NK * CT)
        ).astype(BF16_NP)
        cls = xs[:, 0, :]                                     # [NB, H]
        cls_t = np.ascontiguousarray(
            cls.reshape(NB, NK, P).transpose(2, 1, 0).reshape(P, NK * NB)
        ).astype(np.float32)
        in_maps.append({"x_in": xtd, "cls_t": cls_t, **w_maps})
    return in_maps


def unshard_one(y_np):
    """[P, C*NK*CT] h-major device output -> [NB, L, H] float32."""
    y = np.asarray(y_np, np.float32).reshape(P, C, NK, CT)
    return np.ascontiguousarray(y.transpose(1, 3, 2, 0)).reshape(NB, L, H)


def kernel(x, router_w, lora_down, lora_up):
    nc = get_compiled()
    in_maps = make_in_maps(x, router_w, lora_down, lora_up)
    res = run_bass_kernel_spmd(nc, in_maps, core_ids=list(range(N_CORES)))
    out = np.empty((B, L, H), np.float32)
    for i in range(N_CORES):
        out[i * NB : (i + 1) * NB] = unshard_one(res.results[i]["y_out"])
    return out


# revision 8
# speedup vs baseline: 1.2704x; 1.0839x over previous
"""MoE LoRA adapter layer (top-2 routed, E=8 experts, R=16) on 8 TRN2 NeuronCores.

Strategy: data-parallel over batch B=32 -> 4 batches/core; router + LoRA
weights replicated (tiny). E*R = 128 = partition width, so the per-expert
LoRA down/up projections stack into two dense matmuls:
    P1[er, t] = D_all[er, :] @ x[t, :]^T          (contract H=1024)
    wT[h, t]  = sum_er U_all[er, h] * (gate[b(t), e(er)] * P1[er, t])
The expert sum IS the matmul contraction; gates (exactly 0 off the top-2)
are folded in by scaling P1 columns. out = x + w.

Layout: x is shipped ALREADY TRANSPOSED (h-major) from the host, so the
kernel needs zero PE transposes: MM1 consumes xT directly and MM2 produces
outT in the same h-major layout the store expects. Per chunk (= one batch,
512 tokens) the input slice is one 8 KiB contiguous run per partition.
PSUM->SBUF eviction of the result (the residual add) is split between the
vector and scalar engines so neither becomes the bottleneck. Gates are
computed on-device in fp32 (exact top-2) from a tiny pre-transposed cls.
"""

import sys

if "/opt/trn_rl_repo" not in sys.path:
    sys.path.insert(0, "/opt/trn_rl_repo")

import numpy as np
import ml_dtypes

import concourse.bass as bass
import concourse.tile as tile
from concourse import bacc, mybir
from concourse.bass_utils import run_bass_kernel_spmd

B, L, H = 32, 512, 1024
E, R, TOP_K = 8, 16, 2
N_CORES = 8
NB = B // N_CORES          # batches per core = 4
T = NB * L                 # tokens per core = 2048
P = 128                    # partitions
NK = H // P                # H k-tiles = 8
C = NB                     # chunks per core (one batch = 512 tokens each)
CT = L                     # tokens per chunk

# eviction k-slices that go PSUM->vector directly; the rest take the
# scalar-copy + bf16-add path so scalar shares the eviction load
VEC_DIRECT = (0, 2, 4, 6)

F32 = mybir.dt.float32
BF16 = mybir.dt.bfloat16
BF16_NP = ml_dtypes.bfloat16

_COMPILED = None


def _build():
    """Build + compile the single-core program (same on all 8 cores)."""
    nc = bacc.Bacc("TRN2", target_bir_lowering=False, debug=False)

    x_in = nc.dram_tensor("x_in", [P, C * NK * CT], BF16, kind="ExternalInput")
    cls_t = nc.dram_tensor("cls_t", [P, NK * NB], F32, kind="ExternalInput")
    d_t = nc.dram_tensor("d_t", [P, NK * P], BF16, kind="ExternalInput")
    u_in = nc.dram_tensor("u_in", [P, H], BF16, kind="ExternalInput")
    rwt = nc.dram_tensor("rwt", [P, NK * E], F32, kind="ExternalInput")
    rep = nc.dram_tensor("rep", [E, P], F32, kind="ExternalInput")
    id8 = nc.dram_tensor("id8", [8, 8], F32, kind="ExternalInput")
    y_out = nc.dram_tensor("y_out", [P, C * NK * CT], BF16, kind="ExternalOutput")

    # (p, c, k, t): chunk c, h-tile k, token t -> xT[k*128+p, c*512+t]
    # halves split the chunk by k (0..3 | 4..7): each is 4 KiB/partition
    x_hap = x_in.ap().rearrange("p (c h f) -> c h p f", c=C, h=2)
    # stores go out per (chunk, k-pair): 2 KiB contiguous per partition
    y_ap = y_out.ap().rearrange("p (c g f) -> c g p f", c=C, g=NK // 2)

    with tile.TileContext(nc) as tc:
        with (
            tc.tile_pool(name="wpool", bufs=1) as wpool,
            tc.tile_pool(name="gpool", bufs=1) as gpool,
            tc.tile_pool(name="xpool", bufs=C) as xpool,
            tc.tile_pool(name="opool", bufs=3) as opool,
            tc.tile_pool(name="p2pool", bufs=2) as p2pool,
            tc.tile_pool(name="wbpool", bufs=4) as wbpool,
            tc.tile_pool(name="p1_ps", bufs=2, space="PSUM") as p1_ps,
            tc.tile_pool(name="w_ps", bufs=6, space="PSUM") as w_ps,
        ):
            # ---- loads: ALL on the gpsimd (SWDGE) queue so FIFO order
            # guarantees weights land before the x stream (the v2 trace
            # showed a 256-B identity DMA finishing at t=19us because the
            # x chunks starved it on the shared DMA engines). Order is
            # need-time: gates stuff, d (MM1), x chunk 0, u (MM2), rest.
            cls_sb = gpool.tile([P, NK * NB], F32, tag="cls")
            nc.gpsimd.dma_start(cls_sb[:], cls_t.ap())
            rwt_sb = wpool.tile([P, NK * E], F32, tag="rwt")
            nc.gpsimd.dma_start(rwt_sb[:], rwt.ap())
            id_sb = wpool.tile([8, 8], F32, tag="id8")
            nc.gpsimd.dma_start(id_sb[:], id8.ap())
            rep_sb = wpool.tile([E, P], F32, tag="rep")
            nc.gpsimd.dma_start(rep_sb[:], rep.ap())
            d_sb = wpool.tile([P, NK * P], BF16, tag="d")
            nc.gpsimd.dma_start(d_sb[:], d_t.ap())

            x_tiles = []
            for _c in range(C):
                xb = xpool.tile([P, NK * CT], BF16, tag="xb")
                x_tiles.append(xb)
            u_sb = wpool.tile([P, H], BF16, tag="u")

            def load_half(c, h):
                nc.gpsimd.dma_start(
                    x_tiles[c][:, h * (NK // 2) * CT : (h + 1) * (NK // 2) * CT],
                    x_hap[c, h],
                )

            load_half(0, 0)
            load_half(0, 1)
            nc.gpsimd.dma_start(u_sb[:], u_in.ap())
            for c in range(1, C):
                load_half(c, 0)
                load_half(c, 1)

            holders = {}

            def stage_prologue():
                # logits [NB, E] = cls @ router_w^T, contracted over H
                lg_ps = p1_ps.tile([P, 512], F32, tag="p1")
                for k in range(NK):
                    nc.tensor.matmul(
                        lg_ps[0:NB, 0:E],
                        cls_sb[:, k * NB : (k + 1) * NB],
                        rwt_sb[:, k * E : (k + 1) * E],
                        start=(k == 0),
                        stop=(k == NK - 1),
                    )
                lg = gpool.tile([NB, E], F32, tag="lg")
                nc.vector.tensor_copy(lg[:], lg_ps[0:NB, 0:E])

                # top-2 softmax per row (E=8 along free dim)
                m1 = gpool.tile([NB, 1], F32, tag="m1")
                nc.vector.reduce_max(m1[:], lg[:], axis=mybir.AxisListType.X)
                t_sb = gpool.tile([NB, E], F32, tag="t")
                nc.vector.tensor_scalar(
                    t_sb[:], lg[:], m1[:], None, op0=mybir.AluOpType.subtract
                )
                # pen = (t >= 0) * 1e30  (knocks out the argmax)
                pen = gpool.tile([NB, E], F32, tag="pen")
                nc.vector.tensor_scalar(
                    pen[:], t_sb[:], 0.0, 1e30,
                    op0=mybir.AluOpType.is_ge, op1=mybir.AluOpType.mult,
                )
                t2 = gpool.tile([NB, E], F32, tag="t2")
                nc.vector.tensor_sub(t2[:], t_sb[:], pen[:])
                m2 = gpool.tile([NB, 1], F32, tag="m2")
                nc.vector.reduce_max(m2[:], t2[:], axis=mybir.AxisListType.X)
                keep = gpool.tile([NB, E], F32, tag="keep")
                nc.vector.tensor_scalar(
                    keep[:], t_sb[:], m2[:], None, op0=mybir.AluOpType.is_ge
                )
                ex = gpool.tile([NB, E], F32, tag="ex")
                nc.scalar.activation(ex[:], t_sb[:], mybir.ActivationFunctionType.Exp)
                eg = gpool.tile([NB, E], F32, tag="eg")
                nc.vector.tensor_mul(eg[:], ex[:], keep[:])
                s_sb = gpool.tile([NB, 1], F32, tag="s")
                nc.vector.reduce_sum(s_sb[:], eg[:], axis=mybir.AxisListType.X)
                rs = gpool.tile([NB, 1], F32, tag="rs")
                nc.vector.reciprocal(rs[:], s_sb[:])
                gts = gpool.tile([NB, E], F32, tag="gts")
                nc.vector.tensor_scalar(
                    gts[:], eg[:], rs[:], None, op0=mybir.AluOpType.mult
                )

                # gatesT then replicate x16 along partitions -> gvec [128, NB]
                gt_ps = p1_ps.tile([P, 512], F32, tag="p1")
                nc.tensor.transpose(gt_ps[0:E, 0:NB], gts[:], id_sb[0:NB, 0:NB])
                gtT = gpool.tile([E, NB], F32, tag="gtT")
                nc.vector.tensor_copy(gtT[:], gt_ps[0:E, 0:NB])
                gv_ps = p1_ps.tile([P, 512], F32, tag="p1")
                nc.tensor.matmul(gv_ps[:, 0:NB], rep_sb[:], gtT[:])
                gvec = gpool.tile([P, NB], F32, tag="gvec")
                nc.vector.tensor_copy(gvec[:], gv_ps[:, 0:NB])
                holders["gvec"] = gvec

            p2_tiles = {}

            def stage_mm1(c):
                p1 = p1_ps.tile([P, CT], F32, tag="p1")
                for k in range(NK):
                    nc.tensor.matmul(
                        p1[:],
                        d_sb[:, k * P : (k + 1) * P],
                        x_tiles[c][:, k * CT : (k + 1) * CT],
                        start=(k == 0),
                        stop=(k == NK - 1),
                    )
                p2 = p2pool.tile([P, CT], BF16, tag="p2")
                nc.scalar.activation(
                    p2[:], p1[:], mybir.ActivationFunctionType.Copy,
                    scale=holders["gvec"][:, c : c + 1],
                )
                p2_tiles[c] = p2

            def stage_mm2(c):
                o_sb = opool.tile([P, NK * CT], BF16, tag="o")
                for k in range(NK):
                    wps = w_ps.tile([P, CT], F32, tag="w")
                    nc.tensor.matmul(
                        wps[:], u_sb[:, k * P : (k + 1) * P], p2_tiles[c][:]
                    )
                    x_k = x_tiles[c][:, k * CT : (k + 1) * CT]
                    o_k = o_sb[:, k * CT : (k + 1) * CT]
                    if k in VEC_DIRECT:
                        nc.vector.tensor_add(o_k, wps[:], x_k)
                    else:
                        wb = wbpool.tile([P, CT], BF16, tag="wb")
                        nc.scalar.activation(
                            wb[:], wps[:], mybir.ActivationFunctionType.Copy
                        )
                        nc.vector.tensor_add(o_k, wb[:], x_k)
                    if k % 2 == 1:
                        nc.sync.dma_start(
                            y_ap[c, k // 2], o_sb[:, (k - 1) * CT : (k + 1) * CT]
                        )

            stage_prologue()
            stage_mm1(0)
            for c in range(1, C):
                stage_mm1(c)
                stage_mm2(c - 1)
            stage_mm2(C - 1)

    nc.compile()
    return nc


def _weights_maps(router_w, lora_down, lora_up):
    # D_all[(e,r), h] stacked; lhsT tiles need [p, k, m] = D_all[m, k*128+p]
    d_all = lora_down.reshape(E * R, H)                       # [128, 1024]
    d_t = np.ascontiguousarray(
        d_all.T.reshape(NK, P, E * R).transpose(1, 0, 2).reshape(P, NK * P)
    ).astype(BF16_NP)
    # U_all[(e,r), h] = lora_up[e, h, r]
    u_np = np.ascontiguousarray(
        lora_up.transpose(0, 2, 1).reshape(E * R, H)
    ).astype(BF16_NP)
    # router_wT tiles [p, k, e] = router_w[e, k*128+p]
    rwt_np = np.ascontiguousarray(
        router_w.T.reshape(NK, P, E).transpose(1, 0, 2).reshape(P, NK * E)
    ).astype(np.float32)
    rep_np = np.zeros((E, P), np.float32)
    for e in range(E):
        rep_np[e, e * R : (e + 1) * R] = 1.0
    return {
        "d_t": d_t, "u_in": u_np, "rwt": rwt_np, "rep": rep_np,
        "id8": np.eye(8, dtype=np.float32),
    }


def get_compiled():
    global _COMPILED
    if _COMPILED is None:
        _COMPILED = _build()
    return _COMPILED


def make_in_maps(x, router_w, lora_down, lora_up):
    x = np.asarray(x, np.float32)
    w_maps = _weights_maps(
        np.asarray(router_w, np.float32),
        np.asarray(lora_down, np.float32),
        np.asarray(lora_up, np.float32),
    )
    in_maps = []
    for i in range(N_CORES):
        xs = x[i * NB : (i + 1) * NB]                         # [C, CT, H]
        # (p, c, k, t) <- xs[c, t, k*128+p]
        xtd = np.ascontiguousarray(
            xs.reshape(C, CT, NK, P).transpose(3, 0, 2, 1).reshape(P, C * NK * CT)
        ).astype(BF16_NP)
        cls = xs[:, 0, :]                                     # [NB, H]
        cls_t = np.ascontiguousarray(
            cls.reshape(NB, NK, P).transpose(2, 1, 0).reshape(P, NK * NB)
        ).astype(np.float32)
        in_maps.append({"x_in": xtd, "cls_t": cls_t, **w_maps})
    return in_maps


def unshard_one(y_np):
    """[P, C*NK*CT] h-major device output -> [NB, L, H] float32."""
    y = np.asarray(y_np, np.float32).reshape(P, C, NK, CT)
    return np.ascontiguousarray(y.transpose(1, 3, 2, 0)).reshape(NB, L, H)


def kernel(x, router_w, lora_down, lora_up):
    nc = get_compiled()
    in_maps = make_in_maps(x, router_w, lora_down, lora_up)
    res = run_bass_kernel_spmd(nc, in_maps, core_ids=list(range(N_CORES)))
    out = np.empty((B, L, H), np.float32)
    for i in range(N_CORES):
        out[i * NB : (i + 1) * NB] = unshard_one(res.results[i]["y_out"])
    return out
